# revision 60
# baseline (speedup 1.0000x reference)
"""BiMPM MatchingLayer kernel for Trainium2, 8 NeuronCores, batch-data-parallel.

Full inputs: p (32,64,600), q (32,64,600), W (8,20,300).
Output: tuple (mv_p, mv_q), each (32,64,160).

Per core: 4 batches x 2 directions (fw: cols 0:300, bw: cols 300:600).
Design (567881 ns baseline -> 250430 ns):
- software-pipelined: each iteration's prologue (loads, transposes, norms,
  cosine matrix, replication DMA launches) is emitted one iteration ahead
  of the matching blocks, so inputs get a full iteration of lead time
  (DVE occupancy ~92%).
- p and q rows stacked on 128 partitions (p in 0:64, q in 64:128) so
  per-row ops cover both sides at once.
- bulk elementwise as bf16 tensor_tensor (2x_1p DVE mode, 0.5 cyc/elem);
  per-partition-scalar ops as tensor_scalar (4x bf16); squares and
  per-partition scales on Act; fast approx reciprocals off the
  sign-critical path.
- maxes as bf16 in-place max-trees (tensor_reduce/pool have no fast mode);
  attentive-max tail chunk (h 256:300) packed at base partitions 0/64 and
  multiplied on Pool, feeding extra accumulate matmuls directly.
- matmuls in bf16 (1 cyc/row vs 4 for fp32) except the sign-critical
  cosine matrix (attentive-mean multiplies by sign(sum cos); row sums pass
  within 1e-4 of zero so that path stays fp32 end-to-end).
- partition-broadcasts via DRAM-roundtrip DMA with stride-0 reads.
- W precompute: row-stacked loads, 6 transposes, squaring fused into the
  PSUM copy-out (square commutes with transpose).
- concurrent matmul groups never share a PSUM bank with overlapping
  partition ranges (hardware faults otherwise); accumulation groups end
  with a full-partition-width stop.
"""

import numpy as np

S, H, L, NB, NCORES = 64, 300, 20, 4, 8
CH = [(0, 128), (128, 256), (256, 300)]
WL = 8 * L
AL = 4 * L  # per-direction w-blocks: w in {d, 2+d, 4+d, 6+d}

POOL_MP = True  # maxpool dsc mul on Pool

_CACHE = {}


def _build(nb=NB, en=("fu", "mp", "am", "ax")):
    import concourse.bass as bass
    import concourse.tile as tile
    from concourse import bacc, mybir
    from concourse.masks import make_identity
    from contextlib import ExitStack

    f32 = mybir.dt.float32
    bf16 = mybir.dt.bfloat16
    OP = mybir.AluOpType
    SQ = mybir.ActivationFunctionType.Square

    def V(apx, *dims):
        """Rebuild free dims of a sliced AP: V(slice, [stride,count], ...)."""
        return bass.AP(tensor=apx.tensor, offset=apx.offset,
                       ap=[list(apx.ap[0])] + [list(d) for d in dims])

    nc = bacc.Bacc("TRN2", target_bir_lowering=False, debug=False,
                   enable_asserts=False, num_devices=NCORES)
    p_d = nc.dram_tensor("p", [nb, S, 2 * H], f32, kind="ExternalInput").ap()
    q_d = nc.dram_tensor("q", [nb, S, 2 * H], f32, kind="ExternalInput").ap()
    w_d = nc.dram_tensor("W", [8, L, H], f32, kind="ExternalInput").ap()
    op_d = nc.dram_tensor("op", [nb, S, WL], f32, kind="ExternalOutput").ap()
    oq_d = nc.dram_tensor("oq", [nb, S, WL], f32, kind="ExternalOutput").ap()

    with tile.TileContext(nc) as tc, ExitStack() as ctx:
        const = ctx.enter_context(tc.tile_pool(name="const", bufs=1))
        sb = ctx.enter_context(tc.tile_pool(name="sb", bufs=2))
        sbx = ctx.enter_context(tc.tile_pool(name="sbx", bufs=1))   # big ax scratch
        sbr = ctx.enter_context(tc.tile_pool(name="sbr", bufs=2))   # DMA-rep dst
        sbo = ctx.enter_context(tc.tile_pool(name="sbo", bufs=2))   # OPQ
        ps = ctx.enter_context(tc.tile_pool(name="ps", bufs=3, space="PSUM"))
        psT = ctx.enter_context(tc.tile_pool(name="psT", bufs=2, space="PSUM"))
        psdl = ctx.enter_context(tc.tile_pool(name="psdl", bufs=1, space="PSUM"))
        dram = ctx.enter_context(tc.tile_pool(name="dram", bufs=4, space="DRAM"))

        ident = const.tile([128, 128], f32, tag="ident")
        make_identity(nc, ident)
        onesF = const.tile([128, 128], f32, tag="onesF")
        nc.vector.memset(onesF[:], 1.0)

        # ---- W precompute: vtall[hp, ci, w*L + l] = W[w, l, h0+hp]^2 ----
        vtall = const.tile([128, 3, WL], f32, tag="vtall")
        nc.gpsimd.memset(vtall[:], 0.0)
        # W rows stacked as (w*L, H): 6 transposes instead of 24; square
        # during copy-out (square commutes with transpose)
        wtA = sb.tile([6 * L, H], f32, tag="wtA", bufs=1)
        nc.sync.dma_start(wtA[:], bass.AP(
            tensor=w_d.tensor, offset=w_d.offset, ap=[[H, 6 * L], [1, H]]))
        wtB = sb.tile([2 * L, H], f32, tag="wtB", bufs=1)
        nc.sync.dma_start(wtB[:], bass.AP(
            tensor=w_d.tensor, offset=w_d[6].offset, ap=[[H, 2 * L], [1, H]]))
        for ci, (h0, h1) in enumerate(CH):
            hc = h1 - h0
            pt = psT.tile([128, 128], f32, tag="t")
            nc.tensor.transpose(pt[:hc, 0:6 * L], wtA[:, h0:h1],
                                ident[0:6 * L, 0:6 * L])
            nc.scalar.activation(vtall[:hc, ci, 0:6 * L], pt[:hc, 0:6 * L], SQ)
            pt2 = psT.tile([128, 128], f32, tag="t")
            nc.tensor.transpose(pt2[:hc, 0:2 * L], wtB[:, h0:h1],
                                ident[0:2 * L, 0:2 * L])
            nc.scalar.activation(vtall[:hc, ci, 6 * L:8 * L], pt2[:hc, 0:2 * L],
                                 SQ)
        vtb = const.tile([128, 3, WL], bf16, tag="vtb")
        nc.scalar.copy(vtb[:], vtall[:])
        vtbT = const.tile([128, WL], bf16, tag="vtbT")
        nc.scalar.copy(vtbT[0:S, :], vtb[0:S, 2, :])
        nc.sync.dma_start(vtbT[S:2 * S, :], vtb[0:S, 2, :])
        # vrep[., ci, d, l, q] = W[2+d, l, .]^2 broadcast along q (for mp rhs)
        vrep = const.tile([128, 3, 2, L, S], bf16, tag="vrep")
        for ci in range(3):
            for d in range(2):
                w = 2 + d
                nc.vector.tensor_copy(
                    vrep[:, ci, d],
                    V(vtall[:, ci, w * L:(w + 1) * L], [1, L], [0, S]))

        def vtb_w(ci, hc, w):
            return vtb[:hc, ci, w * L:(w + 1) * L]

        def mpcos_tail(w, kblk, g, yvparts, invnAll, OPQ, sg=None):
            """num = sum_h g*W^2 ; den-inv = invnAll_blk * rsqrt(sum_h yv^2 W^2);
            OPQ[:, w*L:] = num * min(deninv, 1e8) [* sign]."""
            nums = ps.tile([128, 384], f32, tag="t")
            for ci, (h0, h1) in enumerate(CH):
                hc = h1 - h0
                nc.tensor.matmul(nums[:, 0:L], g[:hc, ci, :], vtb_w(ci, hc, w),
                                 start=(ci == 0), stop=(ci == 2))
            y2 = sb.tile([128, 3, 128], bf16, tag="y2")
            for (cc0, cc1, yv) in yvparts:
                nc.scalar.activation(y2[:, :, cc0:cc1], yv, SQ)
            dens = ps.tile([128, 384], f32, tag="t")
            for ci, (h0, h1) in enumerate(CH):
                hc = h1 - h0
                nc.tensor.matmul(dens[:, 0:L], y2[:hc, ci, :], vtb_w(ci, hc, w),
                                 start=(ci == 0), stop=(ci == 2))
            ny = sb.tile([128, L], f32, tag="ny")
            nc.scalar.sqrt(ny[:], dens[:, 0:L])
            invy = sb.tile([128, L], f32, tag="invy")
            nc.vector.reciprocal_approx_fast(invy[:], ny[:])
            c1 = sb.tile([128, L], f32, tag="c1")
            nc.vector.tensor_mul(c1[:], invy[:],
                                 invnAll[:, kblk * L:(kblk + 1) * L])
            if sg is not None:
                c3 = sb.tile([128, L], f32, tag="c3")
                nc.vector.tensor_mul(c3[:], nums[:, 0:L], c1[:])
                nc.vector.tensor_scalar_mul(OPQ[:, w * L:(w + 1) * L], c3[:], sg[:])
            else:
                nc.vector.tensor_mul(OPQ[:, w * L:(w + 1) * L], nums[:, 0:L],
                                     c1[:])

        def maxtree(eng, X0, nmerge, final_out):
            """X0: AP at the start of an (128, nmerge*64) bf16 region.
            In-place max tree over innermost 64; final level -> final_out."""
            n = 64
            while n > 2:
                h = n // 2
                i0 = V(X0, [64, nmerge], [1, h])
                i1 = bass.AP(tensor=X0.tensor, offset=X0.offset + h,
                             ap=[list(X0.ap[0]), [64, nmerge], [1, h]])
                eng.tensor_tensor(i0, i0, i1, OP.max)
                n = h
            i0 = V(X0, [64, nmerge], [1, 1])
            i1 = bass.AP(tensor=X0.tensor, offset=X0.offset + 1,
                         ap=[list(X0.ap[0]), [64, nmerge], [1, 1]])
            eng.tensor_tensor(final_out, i0, i1, OP.max)

        def front(b, d):
            """Per-(b,d) prologue: loads, transposes, norms, cosine matrix,
            replication DMA launches. Emitted one iteration ahead so every
            body op has its inputs long before the engines reach it."""
            F = {"d": d}
            c0 = d * H
            PQ = sb.tile([128, H], f32, tag="PQ", bufs=3, name="PQ")
            nc.sync.dma_start(PQ[0:S, :], p_d[b, :, c0:c0 + H])
            nc.sync.dma_start(PQ[S:2 * S, :], q_d[b, :, c0:c0 + H])
            F["PQ"] = PQ

            TQ = sb.tile([128, 3, 128], f32, tag="TQ", bufs=3, name="TQ")
            nc.gpsimd.memset(TQ[:], 0.0)
            for ci, (h0, h1) in enumerate(CH):
                hc = h1 - h0
                pt = psT.tile([128, 128], f32, tag="t", name="pt")
                nc.tensor.transpose(pt[:hc, :], PQ[:, h0:h1], ident[:, :])
                nc.scalar.copy(TQ[:hc, ci, :], pt[:hc, :])
            TQb = sb.tile([128, 3, 128], bf16, tag="TQb", bufs=3, name="TQb")
            nc.gpsimd.memset(TQb[:], 0.0)
            TQ2 = sb.tile([128, 3, 128], f32, tag="TQ2", bufs=3, name="TQ2")
            for ci, (h0, h1) in enumerate(CH):
                hc = h1 - h0
                nc.scalar.copy(TQb[:hc, ci, :], TQ[:hc, ci, :])
                nc.scalar.activation(TQ2[:hc, ci, :], TQ[:hc, ci, :], SQ)
            F["TQ"], F["TQb"], F["TQ2"] = TQ, TQb, TQ2

            # row norms (precise): nsq via matmul with ones-col
            nsqp = ps.tile([128, 384], f32, tag="t", name="nsqp")
            for ci, (h0, h1) in enumerate(CH):
                hc = h1 - h0
                nc.tensor.matmul(nsqp[:, 0:1], TQ2[:hc, ci, :], onesF[:hc, 0:1],
                                 start=(ci == 0), stop=(ci == 2))
            nP = sb.tile([128, 1], f32, tag="nP", name="nP")
            nc.scalar.sqrt(nP[:], nsqp[:, 0:1])
            invn = sb.tile([128, 1], f32, tag="invn", name="invn")
            nc.vector.reciprocal(invn[:], nP[:])
            invnQ0 = sb.tile([S, 1], f32, tag="invnQ0", name="invnQ0")
            nc.sync.dma_start(invnQ0[:], invn[S:2 * S, :])

            # cosine matrix (fp32, sign-critical): Cs[p,t], Ct[t,p]
            cutp = ps.tile([128, 384], f32, tag="t", name="cutp")
            for ci, (h0, h1) in enumerate(CH):
                hc = h1 - h0
                nc.tensor.matmul(cutp[0:S, 0:S], TQ[:hc, ci, S:2 * S],
                                 TQ[:hc, ci, 0:S],
                                 start=(ci == 0), stop=(ci == 2))
            A = sb.tile([S, S], f32, tag="A", name="A")
            nc.scalar.mul(A[:], cutp[0:S, 0:S], invnQ0[:])
            atp = psT.tile([128, 128], f32, tag="t", name="atp")
            nc.tensor.transpose(atp[0:S, 0:S], A[:], ident[0:S, 0:S])
            Cs = sb.tile([S, S], f32, tag="Cs", name="Cs")
            nc.scalar.mul(Cs[:], atp[0:S, 0:S], invn[0:S, :])
            Csb = sb.tile([S, S], bf16, tag="Csb", name="Csb")
            nc.scalar.copy(Csb[:], Cs[:])
            ctp = psT.tile([128, 128], f32, tag="t", name="ctp")
            nc.tensor.transpose(ctp[0:S, 0:S], Cs[:], ident[0:S, 0:S])
            Ct = sb.tile([S, S], f32, tag="Ct", name="Ct")
            nc.scalar.copy(Ct[:], ctp[0:S, 0:S])
            Ctb = sb.tile([S, S], bf16, tag="Ctb", name="Ctb")
            nc.scalar.copy(Ctb[:], ctp[0:S, 0:S])
            F["Cs"], F["Csb"], F["Ct"], F["Ctb"] = Cs, Csb, Ct, Ctb

            # launch cos replications (consumed by am and ax)
            if "ax" in en or "am" in en:
                ctd = dram.tile([S, S], bf16, tag="ctd", name="ctd")
                nc.sync.dma_start(ctd[:], Ctb[:])
                F["ctd"] = ctd
            if "ax" in en:
                csd = dram.tile([S, S], bf16, tag="csd", name="csd")
                nc.sync.dma_start(csd[:], Csb[:])
                repCs = sbr.tile([128, S * S], bf16, tag="repCs", name="repCs")
                nc.sync.dma_start(repCs[:], bass.AP(
                    tensor=csd.tensor, offset=csd.offset,
                    ap=[[0, 128], [1, S * S]]))
                repCt = sbr.tile([128, S * S], bf16, tag="repCt", name="repCt")
                nc.sync.dma_start(repCt[:], bass.AP(
                    tensor=ctd.tensor, offset=ctd.offset,
                    ap=[[0, 128], [1, S * S]]))
                F["csd"], F["repCs"], F["repCt"] = csd, repCs, repCt
                # packed ci2 inputs + mixed replication (tail path)
                TQin = sb.tile([128, S], bf16, tag="TQin", name="TQin")
                nc.scalar.copy(TQin[0:S, :], TQb[0:S, 2, S:2 * S])
                nc.sync.dma_start(TQin[S:2 * S, :], TQb[0:S, 2, 0:S])
                TQg = sb.tile([128, S], bf16, tag="TQg", name="TQg")
                nc.scalar.copy(TQg[0:S, :], TQb[0:S, 2, 0:S])
                nc.sync.dma_start(TQg[S:2 * S, :], TQb[0:S, 2, S:2 * S])
                repMix = sbr.tile([128, S * S], bf16, tag="repMix", name="repMix")
                nc.sync.dma_start(repMix[0:S, :], bass.AP(
                    tensor=csd.tensor, offset=csd.offset,
                    ap=[[0, S], [1, S * S]]))
                nc.sync.dma_start(repMix[S:2 * S, :], bass.AP(
                    tensor=ctd.tensor, offset=ctd.offset,
                    ap=[[0, S], [1, S * S]]))
                F["TQin"], F["TQg"], F["repMix"] = TQin, TQg, repMix
            if "am" in en:
                ctbhi = sb.tile([128, S], bf16, tag="ctbhi", name="ctbhi")
                nc.sync.dma_start(ctbhi[S:2 * S, :], F["ctd"][:])
                F["ctbhi"] = ctbhi

            # weighted norms for this d's 4 w-blocks: invnAll (128, AL)
            p2v = ps.tile([128, 384], f32, tag="t", name="p2v")
            for ci, (h0, h1) in enumerate(CH):
                hc = h1 - h0
                rhs = bass.AP(tensor=vtall.tensor,
                              offset=vtall[:hc, ci, d * L:d * L + 1].offset,
                              ap=[list(vtall[:hc, ci, 0:1].ap[0]),
                                  [2 * L, 4], [1, L]])
                nc.tensor.matmul(p2v[:, 0:AL], TQ2[:hc, ci, :], rhs,
                                 start=(ci == 0), stop=(ci == 2))
            nAll = sb.tile([128, AL], f32, tag="nAll", name="nAll")
            nc.scalar.sqrt(nAll[:], p2v[:, 0:AL])
            invnAll = sb.tile([128, AL], f32, tag="invnAll", name="invnAll")
            nc.vector.reciprocal_approx_fast(invnAll[:], nAll[:])
            F["invnAll"] = invnAll

            if "fu" in en and d == 0:
                rowT = sb.tile([1, 2 * L], f32, tag="rowT", name="rowT")
                tidx = S - 1
                nc.sync.dma_start(rowT[0:1, 0:L],
                                  invnAll[S + tidx:S + tidx + 1, 0:L])
                nc.sync.dma_start(rowT[0:1, L:2 * L],
                                  invnAll[tidx:tidx + 1, 0:L])
                F["rowT"] = rowT

            # maxpool inverse-norm transposed replications
            if "mp" in en:
                tqp = psT.tile([128, 128], f32, tag="t", name="tqp")
                nc.tensor.transpose(tqp[0:L, :], invnAll[:, L:2 * L], ident[:, :])
                tqb = sb.tile([L, S], bf16, tag="tqb", name="tqb")
                nc.scalar.copy(tqb[:], tqp[0:L, S:2 * S])
                qd_ = dram.tile([L, S], bf16, tag="qd_", name="qd_")
                nc.sync.dma_start(qd_[:], tqb[:])
                tpb = sb.tile([L, S], bf16, tag="tpb", name="tpb")
                nc.scalar.copy(tpb[:], tqp[0:L, 0:S])
                pd_ = dram.tile([L, S], bf16, tag="pd_", name="pd_")
                nc.sync.dma_start(pd_[:], tpb[:])
                invrep = sbr.tile([128, L * S], bf16, tag="invrep", name="invrep")
                nc.sync.dma_start(invrep[0:S, :], bass.AP(
                    tensor=qd_.tensor, offset=qd_.offset,
                    ap=[[0, S], [1, L * S]]))
                nc.sync.dma_start(invrep[S:2 * S, :], bass.AP(
                    tensor=pd_.tensor, offset=pd_.offset,
                    ap=[[0, S], [1, L * S]]))
                F["invrep"] = invrep
            return F

        def body(b, d, F, OPQ):
            PQ, TQ, TQb = F["PQ"], F["TQ"], F["TQb"]
            invnAll = F["invnAll"]

            # ============ FULL matching (w = d, kblk = 0) ============
            if "fu" in en:
                w = d
                tidx = S - 1 if d == 0 else 0
                gfu = sb.tile([128, 3, 128], bf16, tag="gfu", name="gfu")
                for ci in range(3):
                    nc.vector.tensor_scalar_mul(
                        gfu[:, ci, 0:S], TQb[:, ci, 0:S],
                        TQ[:, ci, S + tidx:S + tidx + 1])
                    nc.vector.tensor_scalar_mul(
                        gfu[:, ci, S:2 * S], TQb[:, ci, S:2 * S],
                        TQ[:, ci, tidx:tidx + 1])
                nums = ps.tile([128, 384], f32, tag="t", name="nums")
                for ci, (h0, h1) in enumerate(CH):
                    hc = h1 - h0
                    nc.tensor.matmul(nums[:, 0:L], gfu[:hc, ci, :],
                                     vtb_w(ci, hc, w),
                                     start=(ci == 0), stop=(ci == 2))
                denrep = ps.tile([128, 384], f32, tag="t", name="denrep")
                if tidx == 0:
                    nc.tensor.matmul(denrep[0:S, 0:L], onesF[S:S + 1, 0:S],
                                     invnAll[S:S + 1, 0:L],
                                     start=True, stop=True)
                    nc.tensor.matmul(denrep[S:2 * S, 0:L], onesF[0:1, 0:S],
                                     invnAll[0:1, 0:L],
                                     start=True, stop=True)
                else:
                    rowT = F["rowT"]
                    nc.tensor.matmul(denrep[0:S, 0:L], onesF[0:1, 0:S],
                                     rowT[0:1, 0:L], start=True, stop=True)
                    nc.tensor.matmul(denrep[S:2 * S, 0:L], onesF[0:1, 0:S],
                                     rowT[0:1, L:2 * L], start=True, stop=True)
                c1 = sb.tile([128, L], f32, tag="fc1", name="c1")
                nc.vector.tensor_mul(c1[:], denrep[:, 0:L], invnAll[:, 0:L])
                nc.vector.tensor_mul(OPQ[:, w * L:(w + 1) * L],
                                     nums[:, 0:L], c1[:])

            # ============ MAXPOOL matching (w = 2+d, kblk = 1) ============
            if "mp" in en:
                w = 2 + d
                invrep = F["invrep"]
                rhsQ = sb.tile([128, 3, L, S], bf16, tag="rhsQ", name="rhsQ")
                rhsP = sb.tile([128, 3, L, S], bf16, tag="rhsP", name="rhsP")
                for ci in range(3):
                    nc.vector.tensor_mul(
                        rhsQ[:, ci], V(TQb[:, ci, S:2 * S], [0, L], [1, S]),
                        vrep[:, ci, d])
                    nc.vector.tensor_mul(
                        rhsP[:, ci], V(TQb[:, ci, 0:S], [0, L], [1, S]),
                        vrep[:, ci, d])
                dl = psdl.tile([128, L * S], f32, tag="dl", name="dl")
                for (n0, n1) in ((0, 512), (512, 1024), (1024, 1280)):
                    for ci, (h0, h1) in enumerate(CH):
                        hc = h1 - h0
                        rq = V(rhsQ[:hc, ci, 0:1], [1, L * S])
                        nc.tensor.matmul(dl[0:S, n0:n1],
                                         TQb[:hc, ci, 0:S], rq[:, n0:n1],
                                         start=(ci == 0), stop=(ci == 2))
                    for ci, (h0, h1) in enumerate(CH):
                        hc = h1 - h0
                        rp = V(rhsP[:hc, ci, 0:1], [1, L * S])
                        nc.tensor.matmul(dl[S:2 * S, n0:n1],
                                         TQb[:hc, ci, S:2 * S], rp[:, n0:n1],
                                         start=(ci == 0), stop=(ci == 2))
                dsb = sb.tile([128, L * S], bf16, tag="dsb", name="dsb")
                nc.scalar.copy(dsb[:], dl[:])
                eng = nc.gpsimd if POOL_MP else nc.vector
                dsc = sb.tile([128, L * S], bf16, tag="dsc", name="dsc")
                eng.tensor_mul(dsc[:], dsb[:], invrep[:])
                mx = sb.tile([128, L], bf16, tag="mx", name="mx")
                maxtree(nc.vector, dsc[:, 0:1], L, V(mx[:, 0:1], [1, L], [1, 1]))
                nc.vector.tensor_mul(OPQ[:, w * L:(w + 1) * L], mx[:],
                                     invnAll[:, L:2 * L])

            # ============ ATTENTIVE-MEAN (w = 4+d, kblk = 2) ============
            if "am" in en:
                w = 4 + d
                Cs, Csb, Ct = F["Cs"], F["Csb"], F["Ct"]
                ctbhi = F["ctbhi"]
                PQb = sb.tile([128, H], bf16, tag="PQb", name="PQb")
                nc.scalar.copy(PQb[:], PQ[:])
                rsp = ps.tile([128, 384], f32, tag="t", name="rsp")
                nc.tensor.matmul(rsp[0:S, 0:1], Ct[:], onesF[0:S, 0:1],
                                 start=True, stop=True)
                rsp2 = ps.tile([128, 384], f32, tag="t", name="rsp2")
                nc.tensor.matmul(rsp2[0:S, 0:1], Cs[:], onesF[0:S, 0:1],
                                 start=True, stop=True)
                sg = sb.tile([128, 1], f32, tag="sg", name="sg")
                nc.scalar.sign(sg[0:S, :], rsp[0:S, 0:1])
                sgc = sb.tile([S, 1], f32, tag="sgc", name="sgc")
                nc.scalar.sign(sgc[:], rsp2[0:S, 0:1])
                nc.sync.dma_start(sg[S:2 * S, :], sgc[:])
                yvP = ps.tile([128, 384], f32, tag="t", name="yvP")
                yvQ = ps.tile([128, 384], f32, tag="t", name="yvQ")
                for ci, (h0, h1) in enumerate(CH):
                    hc = h1 - h0
                    nc.tensor.matmul(yvP[:hc, ci * S:(ci + 1) * S],
                                     PQb[S:2 * S, h0:h1], ctbhi[S:2 * S, :],
                                     start=True, stop=True)
                    nc.tensor.matmul(yvQ[:hc, ci * S:(ci + 1) * S],
                                     PQb[0:S, h0:h1], Csb[:],
                                     start=True, stop=True)
                yv = sb.tile([128, 3, 128], bf16, tag="yv", name="yv")
                nc.gpsimd.memset(yv[:], 0.0)
                for ci, (h0, h1) in enumerate(CH):
                    hc = h1 - h0
                    nc.scalar.copy(yv[:hc, ci, 0:S],
                                   yvP[:hc, ci * S:(ci + 1) * S])
                    nc.scalar.copy(yv[:hc, ci, S:2 * S],
                                   yvQ[:hc, ci * S:(ci + 1) * S])
                g = sb.tile([128, 3, 128], bf16, tag="gam", name="g")
                nc.vector.tensor_mul(g[:], TQb[:], yv[:])
                mpcos_tail(w, 2, g, [(0, 2 * S, yv[:])], invnAll, OPQ, sg=sg)

            # ============ ATTENTIVE-MAX (w = 6+d, kblk = 3) ============
            if "ax" in en:
                w = 6 + d
                repCs, repCt = F["repCs"], F["repCt"]
                X2 = sbx.tile([128, 2, 2, S, S], bf16, tag="X2", name="X2")
                nc.vector.tensor_mul(
                    X2[:, 0], V(TQb[:, 0, S:2 * S], [128, 2], [0, S], [1, S]),
                    V(repCs[:, 0:1], [0, 2], [S, S], [1, S]))
                nc.vector.tensor_mul(
                    X2[:, 1], V(TQb[:, 0, 0:S], [128, 2], [0, S], [1, S]),
                    V(repCt[:, 0:1], [0, 2], [S, S], [1, S]))
                ym2 = sb.tile([128, 2, 2, S], bf16, tag="ym2", name="ym2")
                for side in range(2):
                    maxtree(nc.vector, X2[:, side, 0, 0, 0:1], 2 * S,
                            V(ym2[:, side, 0, 0:1], [1, 2 * S], [1, 1]))
                X3 = sbx.tile([128, S, S], bf16, tag="X3", name="X3")
                nc.gpsimd.tensor_mul(
                    X3[:], V(F["TQin"][:, 0:S], [0, S], [1, S]),
                    V(F["repMix"][:, 0:1], [S, S], [1, S]))
                ymT = sb.tile([128, S], bf16, tag="ymT", name="ymT")
                maxtree(nc.vector, X3[:, 0, 0:1], S,
                        V(ymT[:, 0:1], [1, S], [1, 1]))
                gT = sb.tile([128, S], bf16, tag="gT", name="gT")
                nc.vector.tensor_mul(gT[:], F["TQg"][:], ymT[:])
                y2T = sb.tile([128, S], bf16, tag="y2T", name="y2T")
                nc.scalar.activation(y2T[:], ymT[:], SQ)
                g = sb.tile([128, 2, 128], bf16, tag="gax", name="g")
                nc.vector.tensor_mul(g[:, :, 0:S], TQb[:, 0:2, 0:S], ym2[:, 0])
                nc.vector.tensor_mul(g[:, :, S:2 * S], TQb[:, 0:2, S:2 * S],
                                     ym2[:, 1])
                y2 = sb.tile([128, 2, 128], bf16, tag="y2x", name="y2")
                nc.scalar.activation(y2[:, :, 0:S], ym2[:, 0], SQ)
                nc.scalar.activation(y2[:, :, S:2 * S], ym2[:, 1], SQ)
                HT = CH[2][1] - CH[2][0]
                nums = ps.tile([128, 384], f32, tag="t", name="nums")
                dens = ps.tile([128, 384], f32, tag="t", name="dens")
                for (out, lhs2, lhsT) in ((nums, g, gT), (dens, y2, y2T)):
                    nc.tensor.matmul(out[:, 0:L], lhs2[:128, 0, :],
                                     vtb_w(0, 128, w), start=True, stop=False)
                    nc.tensor.matmul(out[0:S, 0:L], lhsT[0:HT, :],
                                     vtbT[0:HT, w * L:(w + 1) * L],
                                     start=False, stop=False)
                    nc.tensor.matmul(out[S:2 * S, 0:L], lhsT[S:S + HT, :],
                                     vtbT[S:S + HT, w * L:(w + 1) * L],
                                     start=False, stop=False)
                    nc.tensor.matmul(out[:, 0:L], lhs2[:128, 1, :],
                                     vtb_w(1, 128, w), start=False, stop=True)
                ny = sb.tile([128, L], f32, tag="ny", name="ny")
                nc.scalar.sqrt(ny[:], dens[:, 0:L])
                invy = sb.tile([128, L], f32, tag="invy", name="invy")
                nc.vector.reciprocal_approx_fast(invy[:], ny[:])
                c1 = sb.tile([128, L], f32, tag="c1", name="c1")
                nc.vector.tensor_mul(c1[:], invy[:], invnAll[:, 3 * L:4 * L])
                nc.vector.tensor_mul(OPQ[:, w * L:(w + 1) * L], nums[:, 0:L],
                                     c1[:])

        # software-pipelined main loop: front(i+1) is emitted before body(i)
        iters = [(b, d) for b in range(nb) for d in range(2)]
        OPQs = {}
        F = front(*iters[0])
        for idx, (b, d) in enumerate(iters):
            if d == 0:
                OPQ = sbo.tile([128, WL], f32, tag="OPQ", name="OPQ")
                if len(en) < 4:
                    nc.vector.memset(OPQ[:], 0.0)
                OPQs[b] = OPQ
            Fn = front(*iters[idx + 1]) if idx + 1 < len(iters) else None
            body(b, d, F, OPQs[b])
            if d == 1:
                nc.sync.dma_start(op_d[b], OPQs[b][0:S, :])
                nc.sync.dma_start(oq_d[b], OPQs[b][S:2 * S, :])
            F = Fn

    nc.compile()
    return nc


def _get_nc(nb=NB, en=("fu", "mp", "am", "ax")):
    key = (nb, tuple(en))
    if key not in _CACHE:
        _CACHE[key] = _build(nb, en)
    return _CACHE[key]


def _run(p, q, W, nb=NB, en=("fu", "mp", "am", "ax"), trace=False):
    from concourse.bass_utils import run_bass_kernel_spmd
    nc = _get_nc(nb, en)
    B = p.shape[0]
    ncores = B // nb
    assert ncores == NCORES and B == nb * NCORES
    in_maps = []
    for c in range(NCORES):
        in_maps.append({
            "p": np.ascontiguousarray(p[c * nb:(c + 1) * nb]),
            "q": np.ascontiguousarray(q[c * nb:(c + 1) * nb]),
            "W": np.ascontiguousarray(W),
        })
    r = run_bass_kernel_spmd(nc, in_maps, core_ids=list(range(NCORES)), trace=trace)
    if trace:
        print("HW exec time:", r.exec_time_ns, "ns")
        print("trace:", r.instructions_and_trace[1] if r.instructions_and_trace else None)
    mv_p = np.concatenate([r.results[c]["op"] for c in range(NCORES)], axis=0)
    mv_q = np.concatenate([r.results[c]["oq"] for c in range(NCORES)], axis=0)
    return mv_p, mv_q


def kernel(p, q, W):
    p = np.asarray(p, dtype=np.float32)
    q = np.asarray(q, dtype=np.float32)
    W = np.asarray(W, dtype=np.float32)
    return _run(p, q, W)
